# revision 31
# baseline (speedup 1.0000x reference)
"""Multi-head attention (B=2,S=2048,E=1024,H=16,DK=DV=64) on 8 Trainium2 cores.

Sharding: core c handles batch c//4 and head-group c%4 (4 heads each).
Each core computes q/k/v projections for its heads, masked softmax attention
(transposed-scores layout), and a partial output projection with its rows of
Wo.  The host sums the 4 partial fp16 outputs per batch and adds bo.

v3: ACT-bound single-chain pipeline with consolidated DMA.
- The scalar engine's exp stream is the critical resource; everything else
  (scores/ctx matmuls, v-proj, output-proj, normalize) is scheduled to keep
  it saturated: scores run one tile ahead through 2 PSUM slots, ctx lags
  behind exp via a deep es buffer, ctx PSUM is evacuated with a single DVE
  copy so the next head's accumulation starts ~1.2us after the last.
- Few large DMAs (2 slabs per x tensor, 1 per weight tensor, mask in 4
  quarter chunks) instead of 119 small ones: the Sync engine's ~0.6us/issue
  serialized the input stream.
- Projections accumulate contraction-outer so matmuls start on the first
  arriving slab; biases fold into the psum->SBUF Identity activations.
"""

import numpy as np
import ml_dtypes

import concourse.bacc as bacc
import concourse.mybir as mybir
import concourse.tile as tile
from concourse import bass_utils

BF = ml_dtypes.bfloat16
dt = mybir.dt

NCORES = 8


def _emit(nc, tc, inp, y_d, S, E, HL, DK):
    EC = E // 128          # contraction chunks for the projections
    NT = S // 128          # seq tiles
    DKL = HL * DK          # local head dims (256)
    NP = DKL // 128        # q/k partition tiles (pairs of heads)
    Exp = mybir.ActivationFunctionType.Exp
    Ident = mybir.ActivationFunctionType.Identity
    CS = 512               # matmul free-dim chunk
    HB = 1024              # attention q0 block
    XJ = 8                 # x slabs per tensor
    XC = EC // XJ          # contraction chunks per slab

    persist = tc.alloc_tile_pool(name="persist", bufs=1)
    qT = [persist.tile([128, S], dt.bfloat16, name=f"qT{m}") for m in range(NP)]
    kT = [persist.tile([128, S], dt.bfloat16, name=f"kT{m}") for m in range(NP)]
    cT = [persist.tile([128, S], dt.bfloat16, name=f"cT{m}") for m in range(NP)]
    vA = [persist.tile([128, HL * (DK + 1)], dt.bfloat16, name=f"vA{t}")
          for t in range(NT)]
    ones = persist.tile([1, S], dt.bfloat16, name="ones")
    nc.gpsimd.memset(ones[:], 1.0)
    neg3 = persist.tile([128, 1], dt.float32, name="neg3")
    nc.gpsimd.memset(neg3[:], -3.0)

    # weights/biases (one DMA each), interleaved into the x stream by need
    w_sb = {nm: persist.tile([128, EC * DKL], dt.bfloat16, name=f"{nm}s")
            for nm in ("wq", "wk", "wv")}
    b_sb = {nm: persist.tile([128, NP], dt.float32, name=f"{nm}s")
            for nm in ("bqp", "bkp")}
    bv_sb = persist.tile([1, DKL], dt.bfloat16, name="bvs")
    wo_sb = persist.tile([128, NP * E], dt.bfloat16, name="wos")

    mask_sb = persist.tile([128, NT * S], dt.bfloat16, name="masks")
    mask3 = mask_sb[:].rearrange("p (t s) -> p t s", t=NT)

    esp = tc.alloc_tile_pool(name="es", bufs=6)
    npool = tc.alloc_tile_pool(name="nrm", bufs=1)
    ysbp = tc.alloc_tile_pool(name="ysb", bufs=2)
    ctxsp = tc.alloc_tile_pool(name="ctxs", bufs=2)
    xsp = tc.alloc_tile_pool(name="xs", bufs=XJ)

    # x slabs: q and k immediately (exp-start depends on kT); xv reuses the
    # xk tag's pool slots, so its DMAs only enter the bandwidth-shared
    # queues once k-proj has consumed xk, and the mask chunks queue behind
    # xv's blocked issue on the in-order Sync engine.
    nc.sync.dma_start(w_sb["wq"][:], inp["wq"])
    nc.sync.dma_start(b_sb["bqp"][:], inp["bqp"])
    xq = []
    for j in range(XJ):
        t_ = xsp.tile([128, XC * S], dt.bfloat16, tag="xq", name=f"xq{j}")
        nc.sync.dma_start(t_[:], inp["xq"][j])
        xq.append(t_)
    nc.sync.dma_start(w_sb["wk"][:], inp["wk"])
    nc.sync.dma_start(b_sb["bkp"][:], inp["bkp"])
    xk = []
    for j in range(XJ):
        t_ = xsp.tile([128, XC * S], dt.bfloat16, tag="xk", name=f"xk{j}")
        nc.sync.dma_start(t_[:], inp["xk"][j])
        xk.append(t_)
    nc.sync.dma_start(w_sb["wv"][:], inp["wv"])
    nc.sync.dma_start(bv_sb[:], inp["bv"][:])
    xv = []
    for j in range(XJ):
        t_ = xsp.tile([128, XC * S], dt.bfloat16, tag="xq", name=f"xv{j}")
        nc.sync.dma_start(t_[:], inp["xv"][j])
        xv.append(t_)
    # first-half mask in 4 chunks
    for g in range(0, NT, 4):
        nc.sync.dma_start(mask3[:, g:g + 4, 0:HB], inp["mask"][:, g:g + 4, 0:HB])
    nc.sync.dma_start(wo_sb[:], inp["wo"])

    # PE warmup: ~5us of dummy matmuls while the input DMA streams in, so
    # the HAM clock-gate is at 8/8 when the projections start.  The scratch
    # operand is never written: garbage values are fine, the result is
    # never read, and zero dependencies means the PE starts immediately.
    wp = tc.alloc_tile_pool(name="warm", bufs=1, space="PSUM")
    warm_ps = wp.tile([128, CS], dt.float32, tag="w", name="warmps")
    for w_i in range(28):
        nc.tensor.matmul(warm_ps[:], qT[0][:, 0:128], qT[0][:, 0:CS],
                         start=(w_i == 0), stop=(w_i == 27))
    wp.release()

    # ---- phase 1: projections, contraction-outer, m=0 pair-tiles first ----
    # exp-start only needs qT[0]/kT[0]; k's m=1 tile drains into attention
    # as background PE work with its psum->SBUF copy on the DVE.
    PROJ = {"wq": (xq, "bqp", 1.0 / np.sqrt(DK), qT),
            "wk": (xk, "bkp", 1.0, kT)}

    def proj_mm(nm, m, n0, ps):
        xt = PROJ[nm][0]
        for c in range(EC):
            nc.tensor.matmul(
                ps[:],
                w_sb[nm][:, c * DKL + 128 * m:c * DKL + 128 * (m + 1)],
                xt[c // XC][:, (c % XC) * S + n0:(c % XC) * S + n0 + CS],
                start=(c == 0), stop=(c == EC - 1))

    mmp = tc.alloc_tile_pool(name="mmps", bufs=4, space="PSUM")
    for nm, m in (("wq", 0), ("wk", 0), ("wq", 1)):
        _, bias_nm, scale, outtiles = PROJ[nm]
        ps = {}
        for n0 in range(0, S, CS):
            ps[n0] = mmp.tile([128, CS], dt.float32, tag="mm",
                              name=f"{nm}ps{m}_{n0}")
            proj_mm(nm, m, n0, ps[n0])
        for n0 in range(0, S, CS):
            nc.scalar.activation(outtiles[m][:, n0:n0 + CS], ps[n0][:],
                                 Ident, bias=b_sb[bias_nm][:, m:m + 1],
                                 scale=scale)
    mmp.release()

    stp = tc.alloc_tile_pool(name="stps", bufs=3, space="PSUM")
    ctxp = tc.alloc_tile_pool(name="ctxps", bufs=1, space="PSUM")

    def km1_unit(n0):
        ps = stp.tile([128, CS], dt.float32, tag="st", name=f"km1_{n0}")
        proj_mm("wk", 1, n0, ps)
        nc.vector.tensor_scalar(kT[1][:, n0:n0 + CS], ps[:], 1.0,
                                b_sb["bkp"][:, 1:2],
                                mybir.AluOpType.mult, mybir.AluOpType.add)

    def vproj_unit(t):
        vps = stp.tile([128, DKL], dt.float32, tag="st", name=f"vps{t}")
        nc.tensor.matmul(vps[:], ones[0:1, 0:128], bv_sb[:],
                         start=True, stop=False)
        for c in range(EC):
            nc.tensor.matmul(
                vps[:],
                xv[c // XC][:, (c % XC) * S + t * 128:(c % XC) * S + (t + 1) * 128],
                w_sb["wv"][:, c * DKL:(c + 1) * DKL],
                start=False, stop=(c == EC - 1))
        nc.gpsimd.memset(vA[t][:], 1.0)
        nc.vector.tensor_copy(
            vA[t][:].rearrange("p (h c) -> p h c", h=HL)[:, :, 0:DK],
            vps[:].rearrange("p (h c) -> p h c", h=HL))

    def yout_unit(s):
        for n0 in range(0, E, CS):
            yps = stp.tile([128, CS], dt.float32, tag="st",
                           name=f"yps{s}_{n0}")
            for p in range(NP):
                nc.tensor.matmul(yps[:], cT[p][:, s * 128:(s + 1) * 128],
                                 wo_sb[:, p * E + n0:p * E + n0 + CS],
                                 start=(p == 0), stop=(p == NP - 1))
            ysb = ysbp.tile([128, CS], dt.float16, tag="ysb",
                            name=f"ysb{s}_{n0}")
            nc.vector.tensor_copy(ysb[:], yps[:])
            nc.sync.dma_start(y_d[s * 128:(s + 1) * 128, n0:n0 + CS], ysb[:])

    bg = []
    bg_hi = 0
    bgn = []
    bgn_hi = 0
    for n0 in range(0, S, CS):
        bg.append(lambda n0=n0: km1_unit(n0))

    # ---- phase 2: attention, software-pipelined emission --------------------
    # Global step stream over (q0, h, t): per step i the emission order is
    #   exp(i), mask-mul(i), scores(i+2), ctx(i-2)
    # so the PE always has the next scores tile in flight while ACT runs
    # exp(i) (no head-of-line blocking behind the mask-dependent ctx), and
    # the ctx accumulation trails two steps (its PSUM slot frees via a
    # single DVE copy right after its block's last ctx matmul).
    steps = [(qi, q0, h, t)
             for qi, q0 in enumerate(range(0, S, HB))
             for h in range(HL)
             for t in range(NT)]
    NS = len(steps)
    st_t = {}
    es_t = {}
    ctx_t = {}

    def scores_emit(i):
        qi, q0, h, t = steps[i]
        pair, sub = h // 2, (h % 2) * 64
        st = stp.tile([128, HB], dt.float32, tag="st", name=f"st{i}")
        for n0 in range(0, HB, CS):
            nc.tensor.matmul(
                st[:, n0:n0 + CS],
                kT[pair][sub:sub + DK, t * 128:(t + 1) * 128],
                qT[pair][sub:sub + DK, q0 + n0:q0 + n0 + CS],
                start=True, stop=True)
        st_t[i] = st

    def ctx_emit(i):
        qi, q0, h, t = steps[i]
        if t == 0:
            ctx_t[h, q0] = ctxp.tile([DK + 1, HB], dt.float32, tag="ctx",
                                     name=f"ctx{h}_{q0}")
        ctx = ctx_t[h, q0]
        es = es_t.pop(i)
        for n0 in range(0, HB, CS):
            nc.tensor.matmul(
                ctx[:, n0:n0 + CS],
                vA[t][:, h * (DK + 1):(h + 1) * (DK + 1)],
                es[:, n0:n0 + CS],
                start=(t == 0), stop=(t == NT - 1))
        if t == NT - 1:
            # free the ctx PSUM banks now; defer the rest of the normalize
            # to the background queue (keeps the DVE queue clear for the
            # mask-muls the exp stream depends on)
            ctx = ctx_t.pop((h, q0))
            ctxs = ctxsp.tile([DK + 1, HB], dt.float16, tag="cs",
                              name=f"cs{h}_{q0}")
            nc.vector.tensor_copy(ctxs[:], ctx[:])
            bgn.append(lambda h=h, q0=q0, ctxs=ctxs: normalize_bg(h, q0, ctxs))

    def normalize_bg(h, q0, ctxs):
        pair, sub = h // 2, (h % 2) * 64
        rc = npool.tile([1, HB], dt.float32, tag="rc", name=f"rc{h}_{q0}")
        nc.vector.tensor_copy(rc[:], ctxs[DK:DK + 1, :])
        nc.vector.reciprocal_approx_fast(rc[:], rc[:])
        bc = npool.tile([DK, HB], dt.float32, tag="bc", name=f"bc{h}_{q0}")
        nc.gpsimd.partition_broadcast(bc[:], rc[:])
        nc.vector.tensor_mul(cT[pair][sub:sub + DK, q0:q0 + HB],
                             ctxs[0:DK, :], bc[:])
        if h == HL - 1:
            # all four heads' cT for this q0 are now emitted; output
            # projection for its s-tiles may enter the PE work queue
            for s in range(q0 // 128, q0 // 128 + HB // 128):
                bg.append(lambda s=s: yout_unit(s))

    scores_emit(0)
    scores_emit(1)
    for i in range(NS):
        qi, q0, h, t = steps[i]
        if i >= 2:
            ctx_emit(i - 2)
        es = esp.tile([128, HB], dt.bfloat16, tag="e", name=f"e{i}")
        nc.scalar.activation(es[:], st_t.pop(i)[:], Exp, bias=neg3[:])
        nc.vector.tensor_mul(es[:], es[:],
                             mask_sb[:, t * S + q0:t * S + q0 + HB])
        es_t[i] = es
        if i + 2 < NS:
            scores_emit(i + 2)
        # second-half mask DMA, spread across (q0=0, h=1)
        if qi == 0 and h == 1 and t % 4 == 0:
            nc.sync.dma_start(mask3[:, t:t + 4, HB:S],
                              inp["mask"][:, t:t + 4, HB:S])
        # v-proj rides just ahead of h0's vA consumption (ctx trails by 2)
        if qi == 0 and h == 0:
            vproj_unit(t)
        # queued PE work (k m=1 proj, output proj) in the PE slack
        elif bg_hi < len(bg) and t % 4 == 1:
            bg[bg_hi]()
            bg_hi += 1
        # deferred normalize chains (DVE/GpSimd, off the mask-mul path)
        if bgn_hi < len(bgn) and t % 4 == 3:
            bgn[bgn_hi]()
            bgn_hi += 1
    ctx_emit(NS - 2)
    ctx_emit(NS - 1)

    while bgn_hi < len(bgn):
        bgn[bgn_hi]()
        bgn_hi += 1
    while bg_hi < len(bg):
        bg[bg_hi]()
        bg_hi += 1

    xsp.release()
    ctxsp.release()
    ysbp.release()
    npool.release()
    esp.release()
    ctxp.release()
    stp.release()
    persist.release()


def _build(S, E, HL, DK):
    EC = E // 128
    NT = S // 128
    DKL = HL * DK
    NP = DKL // 128
    XJ = 8
    XC = EC // XJ
    nc = bacc.Bacc("TRN2", target_bir_lowering=False, debug=False,
                   num_devices=NCORES)
    inp = {}
    for nm in ("xq", "xk", "xv"):
        inp[nm] = nc.dram_tensor(nm, [XJ, 128, XC * S], dt.bfloat16,
                                 kind="ExternalInput").ap()
    for nm in ("wq", "wk", "wv"):
        inp[nm] = nc.dram_tensor(nm, [128, EC * DKL], dt.bfloat16,
                                 kind="ExternalInput").ap()
    for nm in ("bqp", "bkp"):
        inp[nm] = nc.dram_tensor(nm, [128, NP], dt.float32,
                                 kind="ExternalInput").ap()
    inp["bv"] = nc.dram_tensor("bv", [1, DKL], dt.bfloat16,
                               kind="ExternalInput").ap()
    inp["wo"] = nc.dram_tensor("wo", [128, NP * E], dt.bfloat16,
                               kind="ExternalInput").ap()
    inp["mask"] = nc.dram_tensor("mask", [128, NT, S], dt.bfloat16,
                                 kind="ExternalInput").ap()
    y_d = nc.dram_tensor("y", [S, E], dt.float16, kind="ExternalOutput").ap()

    with tile.TileContext(nc) as tc:
        _emit(nc, tc, inp, y_d, S, E, HL, DK)
    nc.compile()
    return nc


_CACHE = {}
_TRACE = False
_TRACE_CORES = (0,)
_LAST_RESULT = None


def _get_nc(S, E, HL, DK):
    key = (S, E, HL, DK)
    if key not in _CACHE:
        _CACHE[key] = _build(S, E, HL, DK)
    return _CACHE[key]


_RUNNER_CACHE = {}


def _get_runner(nc):
    """Cached variant of bass2jax.run_bass_via_pjrt's multi-core path: build
    the jitted shard_map executable once and reuse it across kernel() calls
    (a fresh jax.jit per call re-traces and may recompile)."""
    if id(nc) in _RUNNER_CACHE:
        return _RUNNER_CACHE[id(nc)]
    import jax
    import concourse.mybir as _mybir
    from concourse import bass2jax
    from jax.sharding import Mesh, PartitionSpec
    from jax.experimental.shard_map import shard_map

    bass2jax.install_neuronx_cc_hook()
    pid_name = nc.partition_id_tensor.name if nc.partition_id_tensor else None
    in_names, out_names, out_avals, zero_shapes = [], [], [], []
    for alloc in nc.m.functions[0].allocations:
        if not isinstance(alloc, _mybir.MemoryLocationSet):
            continue
        name = alloc.memorylocations[0].name
        if alloc.kind == "ExternalInput":
            if name != pid_name:
                in_names.append(name)
        elif alloc.kind == "ExternalOutput":
            out_names.append(name)
            shape = tuple(alloc.tensor_shape)
            dtype = _mybir.dt.np(alloc.dtype)
            out_avals.append(jax.core.ShapedArray(shape, dtype))
            zero_shapes.append((shape, dtype))
    n_params = len(in_names)
    n_outs = len(out_avals)
    all_names = in_names + out_names
    if pid_name is not None:
        all_names = all_names + [pid_name]

    def _body(*args):
        operands = list(args)
        if pid_name is not None:
            operands.append(bass2jax.partition_id_tensor())
        return tuple(bass2jax._bass_exec_p.bind(
            *operands,
            out_avals=tuple(out_avals),
            in_names=tuple(all_names),
            out_names=tuple(out_names),
            lowering_input_output_aliases=(),
            sim_require_finite=True,
            sim_require_nnan=True,
            nc=nc,
        ))

    devices = jax.devices()[:NCORES]
    mesh = Mesh(np.asarray(devices), ("core",))
    donate = tuple(range(n_params, n_params + n_outs))
    sharded = jax.jit(
        shard_map(_body, mesh=mesh,
                  in_specs=(PartitionSpec("core"),) * (n_params + n_outs),
                  out_specs=(PartitionSpec("core"),) * n_outs,
                  check_rep=False),
        donate_argnums=donate, keep_unused=True)

    def run(in_maps):
        concat_in = [np.concatenate([np.asarray(m[nm]) for m in in_maps], axis=0)
                     for nm in in_names]
        concat_zeros = [np.zeros((NCORES * s[0], *s[1:]), d)
                        for s, d in zero_shapes]
        outs = sharded(*concat_in, *concat_zeros)
        return [
            {nm: np.asarray(outs[i]).reshape(NCORES, *out_avals[i].shape)[c]
             for i, nm in enumerate(out_names)}
            for c in range(NCORES)
        ]

    _RUNNER_CACHE[id(nc)] = run
    return run


def _slab(xT, EC, S, XJ):
    """[E,S] -> [XJ, 128, (EC//XJ)*S] contraction-chunk-major slabs."""
    XC = EC // XJ
    return np.ascontiguousarray(
        xT.reshape(XJ, XC, 128, S).transpose(0, 2, 1, 3).reshape(
            XJ, 128, XC * S))


def run_sharded(query, key, value, mask, Wq, bq, Wk, bk, Wv, bv, Wo, bo):
    """Full-input -> full-output runner (generic shapes)."""
    global _LAST_RESULT
    query, key, value = (np.asarray(a, np.float32) for a in (query, key, value))
    mask = np.asarray(mask)
    Wq, bq, Wk, bk, Wv, bv, Wo, bo = (
        np.asarray(a, np.float32) for a in (Wq, bq, Wk, bk, Wv, bv, Wo, bo))

    B, S, E = query.shape
    HDK = Wq.shape[1]
    DKv = 64
    H = HDK // DKv
    GPB = NCORES // B                 # cores per batch
    HL = H // GPB                     # heads per core
    DKL = HL * DKv
    NP = DKL // 128
    EC = E // 128
    NT = S // 128
    XJ = 8

    nc = _get_nc(S, E, HL, DKv)

    # per-batch host prep (shared by the 4 cores of a batch)
    xb = {}
    for b in range(B):
        xb[b] = {
            "xq": _slab(query[b].T.astype(BF), EC, S, XJ),
            "xk": _slab(key[b].T.astype(BF), EC, S, XJ),
            "xv": _slab(value[b].T.astype(BF), EC, S, XJ),
            "mask": np.ascontiguousarray(
                mask[b].reshape(S, NT, 128).transpose(2, 1, 0)).astype(BF),
        }

    def _wslab(W, sl):
        # [E, DKL] -> [128, EC*DKL] contraction-chunk-major
        return np.ascontiguousarray(
            W[:, sl].reshape(EC, 128, DKL).transpose(1, 0, 2).reshape(
                128, EC * DKL)).astype(BF)

    in_maps = []
    for c in range(NCORES):
        b, g = c // GPB, c % GPB
        sl = slice(g * DKL, (g + 1) * DKL)
        in_maps.append({
            **xb[b],
            "wq": _wslab(Wq, sl),
            "wk": _wslab(Wk, sl),
            "wv": _wslab(Wv, sl),
            "bqp": np.ascontiguousarray(
                (bq[sl] / np.sqrt(DKv)).astype(np.float32).reshape(NP, 128).T),
            "bkp": np.ascontiguousarray(
                bk[sl].astype(np.float32).reshape(NP, 128).T),
            "bv": bv[sl].astype(BF).reshape(1, DKL),
            "wo": np.ascontiguousarray(
                Wo[sl, :].reshape(NP, 128, E).transpose(1, 0, 2).reshape(
                    128, NP * E)).astype(BF),
        })

    if _TRACE:
        res = bass_utils.run_bass_kernel_spmd(
            nc, in_maps, core_ids=list(range(NCORES)),
            trace=True, trace_cores=list(_TRACE_CORES))
        _LAST_RESULT = res
        results = res.results
    else:
        results = _get_runner(nc)(in_maps)

    y = np.zeros((B, S, E), np.float32)
    for c in range(NCORES):
        y[c // GPB] += results[c]["y"].astype(np.float32)
    y += bo.astype(np.float32)
    return y


def kernel(**inputs):
    return run_sharded(
        inputs["query"], inputs["key"], inputs["value"], inputs["mask"],
        inputs["Wq"], inputs["bq"], inputs["Wk"], inputs["bk"],
        inputs["Wv"], inputs["bv"], inputs["Wo"], inputs["bo"])
